# revision 1
# baseline (speedup 1.0000x reference)
"""Trainium2 Bass kernel for nn_InvNUConv2d: label-grouped 1x1 conv.

  y[b, :, h, w] = weight[labels[b, h, w]] @ x[b, :, h, w] + bias[labels[b, h, w]]

Shapes (hardcoded): x [4, 16, 256, 256] f32, labels [4, 256, 256] i32,
weight [25, 16, 16] f32, bias [25] f32 (zeros).

Sharding: 8 cores, each takes half an image in H: core k -> (b = k//2,
h in [128*(k%2), 128*(k%2)+128)) = 32768 pixels x 16 channels.

Device algorithm per core (channel-major end to end, no transposes):
  - x arrives as [128, 4096] f32: partition (g*16+ch) holds channel ch of
    pixel group g (8 groups of 4096 pixels); pure layout reshape on host.
  - gpsimd.ap_gather groups pixels by label into a fixed slot space of
    25 labels x 256 slots per group (per-16-partition-group indices).
  - one fp32 matmul per label with block-diag(W_l^T x 8) stationary applies
    the label's 16x16 conv to all its pixels at once (K=128 packing).
  - gpsimd.ap_gather with the inverse index map restores raster order.
  - direct DMA out; host undoes the layout reshape.

Host does sharding/layout + index construction from labels; all data
movement and FLOPs run on the NeuronCores.
"""
import numpy as np

import jax
import concourse.bacc as bacc
import concourse.bass as bass  # noqa: F401
import concourse.mybir as mybir
import concourse.tile as tile
from concourse import bass2jax
from jax.sharding import Mesh, PartitionSpec
from jax.experimental.shard_map import shard_map

B, C, H, W, L = 4, 16, 256, 256, 25
N_CORES = 8
NPIX = B * H * W // N_CORES  # 32768 pixels per core
NG = 8                       # partition groups (16 channels each)
GP = NPIX // NG              # 4096 pixels per group
CAP = 256                    # slots per label per group (max count ~220)
M = L * CAP                  # 6400 slots per group
NSPLIT_IN = 4                # ap_gather call splits (pipelining granularity)
NSPLIT_OUT = 2

F32 = mybir.dt.float32
I16 = mybir.dt.int16

assert M % (16 * NSPLIT_IN) == 0 and GP % (16 * NSPLIT_OUT) == 0


def _build_module(iters=1):
    nc = bacc.Bacc("TRN2", target_bir_lowering=False, debug=False, num_devices=N_CORES)
    x8 = nc.dram_tensor("x8", [128, GP], F32, kind="ExternalInput").ap()
    gidx = nc.dram_tensor("gidx", [128, M // 16], I16, kind="ExternalInput").ap()
    yinv = nc.dram_tensor("yinv", [128, GP // 16], I16, kind="ExternalInput").ap()
    wbd = nc.dram_tensor("wbd", [L, 128, 128], F32, kind="ExternalInput").ap()
    y8 = nc.dram_tensor("y8", [128, GP], F32, kind="ExternalOutput").ap()

    with tile.TileContext(nc) as tc:
        with (
            tc.tile_pool(name="persist", bufs=1) as pp,
            tc.tile_pool(name="psum_mm", bufs=4, space="PSUM") as pmp,
        ):
            x_t = pp.tile([128, GP], F32)
            gidx_t = pp.tile([128, M // 16], I16)
            yinv_t = pp.tile([128, GP // 16], I16)
            wbd_t = pp.tile([128, L * 128], F32)
            xs = pp.tile([128, M], F32)
            ys = pp.tile([128, M], F32)
            yo = pp.tile([128, GP], F32)
            nc.sync.dma_start(x_t[:], x8[:])
            nc.sync.dma_start(gidx_t[:], gidx[:])
            nc.sync.dma_start(yinv_t[:], yinv[:])
            for l in range(L):
                nc.sync.dma_start(wbd_t[:, l * 128 : (l + 1) * 128], wbd[l])

            for _ in range(iters):
                # phase 1: group pixels by label into slot space
                ksp = M // NSPLIT_IN
                for s in range(NSPLIT_IN):
                    nc.gpsimd.ap_gather(
                        out_ap=xs[:, s * ksp : (s + 1) * ksp],
                        in_ap=x_t[:],
                        idxs_ap=gidx_t[:, s * (ksp // 16) : (s + 1) * (ksp // 16)],
                        channels=128,
                        num_elems=GP,
                        d=1,
                        num_idxs=ksp,
                    )
                # phase 2: one matmul per label (block-diag W^T stationary)
                for l in range(L):
                    pm = pmp.tile([128, CAP], F32, tag="pm")
                    nc.tensor.matmul(
                        out=pm[:],
                        lhsT=wbd_t[:, l * 128 : (l + 1) * 128],
                        rhs=xs[:, l * CAP : (l + 1) * CAP],
                        start=True,
                        stop=True,
                    )
                    nc.vector.tensor_copy(out=ys[:, l * CAP : (l + 1) * CAP], in_=pm[:])
                # phase 3: inverse gather back to raster order
                ksp2 = GP // NSPLIT_OUT
                for s in range(NSPLIT_OUT):
                    nc.gpsimd.ap_gather(
                        out_ap=yo[:, s * ksp2 : (s + 1) * ksp2],
                        in_ap=ys[:],
                        idxs_ap=yinv_t[:, s * (ksp2 // 16) : (s + 1) * (ksp2 // 16)],
                        channels=128,
                        num_elems=M,
                        d=1,
                        num_idxs=ksp2,
                    )
            nc.sync.dma_start(y8[:], yo[:])
    nc.compile()
    return nc


def _make_runner(nc):
    bass2jax.install_neuronx_cc_hook()
    partition_name = nc.partition_id_tensor.name if nc.partition_id_tensor else None
    in_names, out_names, out_avals, zero_outs = [], [], [], []
    for alloc in nc.m.functions[0].allocations:
        if not isinstance(alloc, mybir.MemoryLocationSet):
            continue
        name = alloc.memorylocations[0].name
        if alloc.kind == "ExternalInput":
            if name != partition_name:
                in_names.append(name)
        elif alloc.kind == "ExternalOutput":
            shape = tuple(alloc.tensor_shape)
            dtype = mybir.dt.np(alloc.dtype)
            out_names.append(name)
            out_avals.append(jax.core.ShapedArray(shape, dtype))
            zero_outs.append(np.zeros(shape, dtype))
    n_params = len(in_names)
    in_names_full = in_names + out_names + ([partition_name] if partition_name else [])

    def _body(*args):
        operands = list(args)
        if partition_name is not None:
            operands.append(bass2jax.partition_id_tensor())
        outs = bass2jax._bass_exec_p.bind(
            *operands,
            out_avals=tuple(out_avals),
            in_names=tuple(in_names_full),
            out_names=tuple(out_names),
            lowering_input_output_aliases=(),
            sim_require_finite=False,
            sim_require_nnan=False,
            nc=nc,
        )
        return tuple(outs)

    devices = jax.devices()[:N_CORES]
    mesh = Mesh(np.asarray(devices), ("core",))
    nin = n_params + len(out_names)
    sharded = jax.jit(
        shard_map(
            _body,
            mesh=mesh,
            in_specs=(PartitionSpec("core"),) * nin,
            out_specs=(PartitionSpec("core"),) * len(out_names),
            check_rep=False,
        ),
        keep_unused=True,
    )

    def run(in_maps):
        per_core = [[np.asarray(m[name]) for name in in_names] for m in in_maps]
        concat_in = [
            np.concatenate([per_core[c][i] for c in range(N_CORES)], axis=0)
            for i in range(n_params)
        ]
        concat_zeros = [
            np.zeros((N_CORES * z.shape[0], *z.shape[1:]), z.dtype) for z in zero_outs
        ]
        out_arrs = sharded(*concat_in, *concat_zeros)
        out_arrs = [np.asarray(a) for a in out_arrs]
        return [
            {
                name: out_arrs[i].reshape(N_CORES, *out_avals[i].shape)[c]
                for i, name in enumerate(out_names)
            }
            for c in range(N_CORES)
        ]

    return run


_CACHE = {}


def _get_runner(iters=1):
    if iters not in _CACHE:
        nc = _build_module(iters)
        _CACHE[iters] = _make_runner(nc)
    return _CACHE[iters]


def _wrap16(logical):
    """[NG, n] logical per-group index list -> [128, n//16] wrapped layout."""
    ng, n = logical.shape
    out = np.zeros((128, n // 16), np.int16)
    for g in range(ng):
        out[g * 16 : (g + 1) * 16, :] = logical[g].reshape(n // 16, 16).T
    return out


def _prep_core_inputs(xc, lc, weight):
    """xc [C, 128, W] f32, lc [128, W] i32 -> per-core input dict."""
    x_flat = np.asarray(xc).reshape(C, NPIX)  # pixel = h_local*W + w
    x8 = np.ascontiguousarray(
        x_flat.reshape(C, NG, GP).transpose(1, 0, 2).reshape(128, GP)
    )
    lab = np.asarray(lc).reshape(NPIX)

    gidx_log = np.zeros((NG, M), np.int16)  # slot -> local pixel (pads -> 0)
    yinv_log = np.zeros((NG, GP), np.int16)  # local pixel -> slot
    for g in range(NG):
        lg = lab[g * GP : (g + 1) * GP]
        order = np.argsort(lg, kind="stable")
        counts = np.bincount(lg, minlength=L)
        if counts.max() > CAP:
            raise RuntimeError(f"label count {counts.max()} exceeds CAP={CAP}")
        pos = 0
        for l in range(L):
            n = counts[l]
            pix = order[pos : pos + n]
            gidx_log[g, l * CAP : l * CAP + n] = pix
            yinv_log[g, pix] = l * CAP + np.arange(n, dtype=np.int16)
            pos += n

    wbd = np.zeros((L, 128, 128), np.float32)
    for l in range(L):
        wt = weight[l].T  # lhsT[(g,ch),(g,o)] = W[l, o, ch]
        for g in range(NG):
            wbd[l, g * 16 : g * 16 + 16, g * 16 : g * 16 + 16] = wt
    return {
        "x8": x8,
        "gidx": _wrap16(gidx_log),
        "yinv": _wrap16(yinv_log),
        "wbd": wbd,
    }


def kernel(x, labels, weight, bias):
    x = np.asarray(x, dtype=np.float32)
    labels = np.asarray(labels, dtype=np.int32)
    weight = np.asarray(weight, dtype=np.float32)
    bias = np.asarray(bias, dtype=np.float32)

    run = _get_runner(1)
    in_maps = []
    for k in range(N_CORES):
        b, hh = k // 2, (k % 2) * 128
        in_maps.append(
            _prep_core_inputs(x[b, :, hh : hh + 128, :], labels[b, hh : hh + 128, :], weight)
        )
    res = run(in_maps)

    y = np.empty((B, C, H, W), dtype=np.float32)
    for k in range(N_CORES):
        b, hh = k // 2, (k % 2) * 128
        yk = res[k]["y8"].reshape(NG, C, GP).transpose(1, 0, 2).reshape(C, 128, W)
        y[b, :, hh : hh + 128, :] = yk
    if np.any(bias):
        y += bias[labels][:, None, :, :]
    return y



# revision 2
# speedup vs baseline: 32.7117x; 32.7117x over previous
"""Trainium2 Bass kernel for nn_InvNUConv2d: label-grouped 1x1 conv.

  y[b, :, h, w] = weight[labels[b, h, w]] @ x[b, :, h, w] + bias[labels[b, h, w]]

Shapes (hardcoded): x [4, 16, 256, 256] f32, labels [4, 256, 256] i32,
weight [25, 16, 16] f32, bias [25] f32 (zeros).

Sharding: 8 cores, each takes half an image in H: core k -> (b = k//2,
h in [128*(k%2), 128*(k%2)+128)) = 32768 pixels x 16 channels.

Device algorithm per core (bf16 data path; rel-err ~4e-3 << 2e-2 gate):
  - x arrives as [128, 4096] bf16: partition (g*16+ch) holds channel ch of
    pixel group g (8 groups of 4096 pixels); pure layout reshape on host.
  - gpsimd.local_scatter groups pixels by label into a fixed slot space of
    25 labels x CAP slots per group (per-partition indices; dst chunked to
    <=2046 columns by the 64KB GPSIMD scratch limit, chunks label-aligned
    so matmuls of chunk c overlap the scatter of chunk c+1).
  - one bf16 matmul per label with block-diag(W_l^T x 8) stationary applies
    the label's 16x16 conv to all its pixels at once (K=128 packing).
  - gpsimd.local_scatter with the inverse map restores raster order
    (pad slots carry idx=-1 and are dropped).
  - direct DMA out; host undoes the layout reshape.

local_scatter (~2.6ns per visited column) replaces the previous
ap_gather (~22ns/idx) as the permutation engine: Pool-engine time drops
from ~236us to ~75us per iteration.

Host does sharding/layout + index construction from labels; all data
movement and FLOPs run on the NeuronCores.
"""
import numpy as np
import ml_dtypes

import jax
import concourse.bacc as bacc
import concourse.bass as bass  # noqa: F401
import concourse.mybir as mybir
import concourse.tile as tile
from concourse import bass2jax
from jax.sharding import Mesh, PartitionSpec
from jax.experimental.shard_map import shard_map

B, C, H, W, L = 4, 16, 256, 256, 25
N_CORES = 8
NPIX = B * H * W // N_CORES  # 32768 pixels per core
NG = 8                       # partition groups (16 channels each)
GP = NPIX // NG              # 4096 pixels per group
CAP = 224                    # slots per label per group (input max is 210)
M = L * CAP                  # slots per group
MAX_ELEMS = 2046             # local_scatter dst columns per instruction

F32 = mybir.dt.float32
BF16 = mybir.dt.bfloat16
I16 = mybir.dt.int16
BF16_NP = ml_dtypes.bfloat16


def _chunks_label_aligned(total, cap_per_label):
    """Split [0, total) into dst chunks <= MAX_ELEMS, label-aligned."""
    per = (MAX_ELEMS // cap_per_label) * cap_per_label
    out = []
    s = 0
    while s < total:
        ln = min(per, total - s)
        out.append((s, ln))
        s += ln
    return out


def _chunks_even(total):
    n = -(-total // MAX_ELEMS)
    base = total // n
    if base % 2:
        base -= 1
    out = []
    s = 0
    for i in range(n):
        ln = base if i < n - 1 else total - s
        out.append((s, ln))
        s += ln
    assert all(ln % 2 == 0 and ln <= MAX_ELEMS for _, ln in out)
    return out


P1_CHUNKS = _chunks_label_aligned(M, CAP)   # slot-space chunks
P3_CHUNKS = _chunks_even(GP)                # raster chunks


def _build_module(iters=1):
    nc = bacc.Bacc("TRN2", target_bir_lowering=False, debug=False, num_devices=N_CORES)
    x8 = nc.dram_tensor("x8", [128, GP], BF16, kind="ExternalInput").ap()
    p1i = [
        nc.dram_tensor(f"p1i{c}", [128, GP], I16, kind="ExternalInput").ap()
        for c in range(len(P1_CHUNKS))
    ]
    p3i = [
        nc.dram_tensor(f"p3i{c}", [128, M], I16, kind="ExternalInput").ap()
        for c in range(len(P3_CHUNKS))
    ]
    wbd = nc.dram_tensor("wbd", [L, 128, 128], BF16, kind="ExternalInput").ap()
    y8 = nc.dram_tensor("y8", [128, GP], BF16, kind="ExternalOutput").ap()

    with tile.TileContext(nc) as tc:
        with (
            tc.tile_pool(name="persist", bufs=1) as pp,
            tc.tile_pool(name="psum_mm", bufs=4, space="PSUM") as pmp,
        ):
            x_t = pp.tile([128, GP], BF16)
            p1_t = [pp.tile([128, GP], I16, name=f"p1t{c}") for c in range(len(P1_CHUNKS))]
            p3_t = [pp.tile([128, M], I16, name=f"p3t{c}") for c in range(len(P3_CHUNKS))]
            wbd_t = pp.tile([128, L * 128], BF16)
            xs = pp.tile([128, M], BF16)
            ys = pp.tile([128, M], BF16)
            yo = pp.tile([128, GP], BF16)
            nc.sync.dma_start(x_t[:], x8[:])
            for c in range(len(P1_CHUNKS)):
                nc.sync.dma_start(p1_t[c][:], p1i[c][:])
            for c in range(len(P3_CHUNKS)):
                nc.sync.dma_start(p3_t[c][:], p3i[c][:])
            for l in range(L):
                nc.sync.dma_start(wbd_t[:, l * 128 : (l + 1) * 128], wbd[l])

            for _ in range(iters):
                # phase 1: scatter raster pixels into label-grouped slots
                for c, (base, ln) in enumerate(P1_CHUNKS):
                    nc.gpsimd.local_scatter(
                        out_ap=xs[:, base : base + ln],
                        data_ap=x_t[:],
                        idxs_ap=p1_t[c][:],
                        channels=128,
                        num_elems=ln,
                        num_idxs=GP,
                    )
                # phase 2: one matmul per label (block-diag W^T stationary)
                for l in range(L):
                    pm = pmp.tile([128, CAP], F32, tag="pm")
                    nc.tensor.matmul(
                        out=pm[:],
                        lhsT=wbd_t[:, l * 128 : (l + 1) * 128],
                        rhs=xs[:, l * CAP : (l + 1) * CAP],
                        start=True,
                        stop=True,
                    )
                    nc.vector.tensor_copy(out=ys[:, l * CAP : (l + 1) * CAP], in_=pm[:])
                # phase 3: scatter slots back to raster order (pads idx=-1)
                for c, (base, ln) in enumerate(P3_CHUNKS):
                    nc.gpsimd.local_scatter(
                        out_ap=yo[:, base : base + ln],
                        data_ap=ys[:],
                        idxs_ap=p3_t[c][:],
                        channels=128,
                        num_elems=ln,
                        num_idxs=M,
                    )
            nc.sync.dma_start(y8[:], yo[:])
    nc.compile()
    return nc


def _make_runner(nc):
    bass2jax.install_neuronx_cc_hook()
    partition_name = nc.partition_id_tensor.name if nc.partition_id_tensor else None
    in_names, out_names, out_avals, zero_outs = [], [], [], []
    for alloc in nc.m.functions[0].allocations:
        if not isinstance(alloc, mybir.MemoryLocationSet):
            continue
        name = alloc.memorylocations[0].name
        if alloc.kind == "ExternalInput":
            if name != partition_name:
                in_names.append(name)
        elif alloc.kind == "ExternalOutput":
            shape = tuple(alloc.tensor_shape)
            dtype = mybir.dt.np(alloc.dtype)
            out_names.append(name)
            out_avals.append(jax.core.ShapedArray(shape, dtype))
            zero_outs.append(np.zeros(shape, dtype))
    n_params = len(in_names)
    in_names_full = in_names + out_names + ([partition_name] if partition_name else [])

    def _body(*args):
        operands = list(args)
        if partition_name is not None:
            operands.append(bass2jax.partition_id_tensor())
        outs = bass2jax._bass_exec_p.bind(
            *operands,
            out_avals=tuple(out_avals),
            in_names=tuple(in_names_full),
            out_names=tuple(out_names),
            lowering_input_output_aliases=(),
            sim_require_finite=False,
            sim_require_nnan=False,
            nc=nc,
        )
        return tuple(outs)

    devices = jax.devices()[:N_CORES]
    mesh = Mesh(np.asarray(devices), ("core",))
    nin = n_params + len(out_names)
    sharded = jax.jit(
        shard_map(
            _body,
            mesh=mesh,
            in_specs=(PartitionSpec("core"),) * nin,
            out_specs=(PartitionSpec("core"),) * len(out_names),
            check_rep=False,
        ),
        keep_unused=True,
    )

    def run(in_maps):
        per_core = [[np.asarray(m[name]) for name in in_names] for m in in_maps]
        concat_in = [
            np.concatenate([per_core[c][i] for c in range(N_CORES)], axis=0)
            for i in range(n_params)
        ]
        concat_zeros = [
            np.zeros((N_CORES * z.shape[0], *z.shape[1:]), z.dtype) for z in zero_outs
        ]
        out_arrs = sharded(*concat_in, *concat_zeros)
        out_arrs = [np.asarray(a) for a in out_arrs]
        return [
            {
                name: out_arrs[i].reshape(N_CORES, *out_avals[i].shape)[c]
                for i, name in enumerate(out_names)
            }
            for c in range(N_CORES)
        ]

    return run


_CACHE = {}


def _get_runner(iters=1):
    if iters not in _CACHE:
        nc = _build_module(iters)
        _CACHE[iters] = _make_runner(nc)
    return _CACHE[iters]


def _prep_core_inputs(xc, lc, weight):
    """xc [C, 128, W] f32, lc [128, W] i32 -> per-core input dict."""
    x_flat = np.asarray(xc).reshape(C, NPIX)  # pixel = h_local*W + w
    x8 = np.ascontiguousarray(
        x_flat.reshape(C, NG, GP).transpose(1, 0, 2).reshape(128, GP)
    ).astype(BF16_NP)
    lab = np.asarray(lc).reshape(NPIX)

    slot_of_pix = np.zeros((NG, GP), np.int16)   # local pixel -> slot
    raster_of_slot = np.full((NG, M), -1, np.int16)  # slot -> local pixel (pads -1)
    for g in range(NG):
        lg = lab[g * GP : (g + 1) * GP]
        order = np.argsort(lg, kind="stable")
        counts = np.bincount(lg, minlength=L)
        if counts.max() > CAP:
            raise RuntimeError(f"label count {counts.max()} exceeds CAP={CAP}")
        starts = np.zeros(L, np.int64)
        starts[1:] = np.cumsum(counts)[:-1]
        rank = np.empty(GP, np.int64)
        rank[order] = np.arange(GP) - np.repeat(starts, counts)
        slots = (lab[g * GP : (g + 1) * GP].astype(np.int64) * CAP + rank).astype(
            np.int16
        )
        slot_of_pix[g] = slots
        raster_of_slot[g, slots] = np.arange(GP, dtype=np.int16)

    out = {"x8": x8}
    for c, (base, ln) in enumerate(P1_CHUNKS):
        rel = slot_of_pix.astype(np.int32) - base
        rel[(rel < 0) | (rel >= ln)] = -1
        out[f"p1i{c}"] = np.repeat(rel.astype(np.int16), 16, axis=0)
    for c, (base, ln) in enumerate(P3_CHUNKS):
        rel = raster_of_slot.astype(np.int32) - base
        rel[(raster_of_slot < 0) | (rel < 0) | (rel >= ln)] = -1
        out[f"p3i{c}"] = np.repeat(rel.astype(np.int16), 16, axis=0)

    wbd = np.zeros((L, 128, 128), np.float32)
    for l in range(L):
        wt = weight[l].T  # lhsT[(g,ch),(g,o)] = W[l, o, ch]
        for g in range(NG):
            wbd[l, g * 16 : g * 16 + 16, g * 16 : g * 16 + 16] = wt
    out["wbd"] = wbd.astype(BF16_NP)
    return out


def kernel(x, labels, weight, bias):
    x = np.asarray(x, dtype=np.float32)
    labels = np.asarray(labels, dtype=np.int32)
    weight = np.asarray(weight, dtype=np.float32)
    bias = np.asarray(bias, dtype=np.float32)

    run = _get_runner(1)
    in_maps = []
    for k in range(N_CORES):
        b, hh = k // 2, (k % 2) * 128
        in_maps.append(
            _prep_core_inputs(x[b, :, hh : hh + 128, :], labels[b, hh : hh + 128, :], weight)
        )
    res = run(in_maps)

    y = np.empty((B, C, H, W), dtype=np.float32)
    for k in range(N_CORES):
        b, hh = k // 2, (k % 2) * 128
        yk = (
            res[k]["y8"]
            .astype(np.float32)
            .reshape(NG, C, GP)
            .transpose(1, 0, 2)
            .reshape(C, 128, W)
        )
        y[b, :, hh : hh + 128, :] = yk
    if np.any(bias):
        y += bias[labels][:, None, :, :]
    return y
